# revision 3
# baseline (speedup 1.0000x reference)
"""MoE feed-forward (top-2 routing, E=8 experts) on 8 trn2 NeuronCores.

Strategy: expert parallelism (1 expert per core).
  - Router is token-sharded: core i routes tokens [1024*i, 1024*(i+1)) and the
    per-token metadata (top-2 expert ids + gate weights) is AllGather'd.
  - Every core then (redundantly, SPMD-uniform) computes per-expert ranks via
    cumsum (triangular matmul + scan), builds the global combine-index table,
    and compacts its OWN expert's token list with indirect-DMA scatters.
  - Dispatch: indirect-DMA gather of x rows from a full replica of x.
  - Expert FFN: h = gelu(x @ w1 + b1); o = (h @ w2 + b2) * gate, in fp32r
    matmuls (full PE rate) with fp32 accumulate.
  - Combine: AllGather of all expert outputs, then each core gathers its own
    tokens' two expert rows and adds them.

Token layout on-chip: [128 partitions, 64 columns], token t = 128*c + p.
Slot layout (per expert, capacity 2560): slot r lives at SBUF (p=r%128, s=r//128).
"""
import numpy as np

import concourse.tile as tile
from concourse import bass, bacc, mybir
from concourse.bass_utils import run_bass_kernel_spmd
from concourse.masks import make_identity, make_upper_triangular

N_CORES = 8
P = 128
E = 8
K = 2
D = 1024
F = 2048
B, S = 4, 2048
T = B * S                  # 8192 tokens
TPC = T // N_CORES         # 1024 tokens per core
CAP = 2560                 # ceil(1.25 * T * K / E)
NSLOT_T = CAP // P         # 20 slot tiles
NCOL = T // P              # 64 token columns
GRP = 512                  # moving free dim per matmul group
NGRP = CAP // GRP          # 5 groups
DC = D // P                # 8 d-chunks
FC = F // P                # 16 f-chunks
DUMMY_ROW = E * CAP        # zero row in the gathered expert-output table
f32 = mybir.dt.float32
f32r = mybir.dt.float32r
i32 = mybir.dt.int32


def build_kernel():
    nc = bacc.Bacc(num_devices=N_CORES)

    # ---------------- parameters ----------------
    x_full = nc.declare_dram_parameter("x_full", [T, D], f32, isOutput=False)
    x_shard = nc.declare_dram_parameter("x_shard", [TPC, D], f32, isOutput=False)
    rw = nc.declare_dram_parameter("rw", [D, E], f32, isOutput=False)
    rb_b = nc.declare_dram_parameter("rb_b", [P, E], f32, isOutput=False)
    w1_p = nc.declare_dram_parameter("w1_p", [D, F], f32, isOutput=False)
    b1_p = nc.declare_dram_parameter("b1_p", [P, FC], f32, isOutput=False)
    w2_p = nc.declare_dram_parameter("w2_p", [F, D], f32, isOutput=False)
    b2_p = nc.declare_dram_parameter("b2_p", [P, DC], f32, isOutput=False)
    my_e = nc.declare_dram_parameter("my_e", [P, 1], f32, isOutput=False)
    my_rows = nc.declare_dram_parameter("my_rows", [P, TPC // P], i32, isOutput=False)
    out_shard = nc.declare_dram_parameter("out_shard", [TPC, D], f32, isOutput=True)

    # ---------------- internal DRAM ----------------
    meta_in = nc.dram_tensor("meta_in", [P, 4 * E], f32)            # my 8 cols x 4 fields
    meta_all = nc.dram_tensor("meta_all", [N_CORES, P, 4 * E], f32, addr_space="Shared")
    slotmeta = nc.dram_tensor("slotmeta", [CAP, 2], f32)            # (tok, gate) by remapped slot
    gidx_d = nc.dram_tensor("gidx_d", [P, 2 * NCOL], i32)           # combine indices, p-major
    eo_d = nc.dram_tensor("eo_d", [CAP, D], f32)                    # my expert outputs by slot
    eo_all = nc.dram_tensor("eo_all", [E * CAP + P, D], f32, addr_space="Shared")

    with tile.TileContext(nc) as tc:
        # =========== phase pools (routing) ===========
        with tc.tile_pool(name="const", bufs=1) as cpool:
            ident = cpool.tile([P, P], f32)
            make_identity(nc, ident[:])
            tri = cpool.tile([P, P], f32)
            make_upper_triangular(nc, tri[:], val=1.0, diag=False)  # tri[p,i]=1 iff p<i
            ones_col = cpool.tile([P, 1], f32)
            nc.gpsimd.memset(ones_col[:], 1.0)
            ones_row1 = cpool.tile([1, P], f32)
            nc.gpsimd.memset(ones_row1[:], 1.0)
            rb_sb = cpool.tile([P, E], f32)
            nc.sync.dma_start(out=rb_sb[:], in_=rb_b.ap())
            mye_sb = cpool.tile([P, 1], f32)
            nc.sync.dma_start(out=mye_sb[:], in_=my_e.ap())
            rw_sb = cpool.tile([P, DC, E], f32)
            nc.sync.dma_start(out=rw_sb[:], in_=rw.ap().rearrange("(c p) e -> p c e", p=P))
            tokf = cpool.tile([P, NCOL], f32)
            toki = cpool.tile([P, NCOL], i32)
            nc.gpsimd.iota(toki[:], pattern=[[P, NCOL]], base=0, channel_multiplier=1)
            nc.vector.tensor_copy(tokf[:], toki[:])

            # ---------- router on my shard ----------
            with tc.tile_pool(name="rt", bufs=2) as rt, \
                 tc.tile_pool(name="rtp", bufs=2, space="PSUM") as rtp:
                meta_sb = rt.tile([P, 4 * E], f32, bufs=1)
                for g in range(TPC // P):
                    xs = rt.tile([P, D], f32, tag="xs")
                    nc.sync.dma_start(out=xs[:], in_=x_shard.ap()[g * P:(g + 1) * P, :])
                    xT = rt.tile([P, DC, P], f32, tag="xT")
                    for dci in range(DC):
                        tp = rtp.tile([P, P], f32, space="PSUM", tag="tp")
                        nc.tensor.transpose(out=tp[:], in_=xs[:, dci * P:(dci + 1) * P],
                                            identity=ident[:])
                        nc.vector.tensor_copy(xT[:, dci, :], tp[:])
                    lg = rtp.tile([P, E], f32, space="PSUM", tag="lg")
                    for dci in range(DC):
                        nc.tensor.matmul(out=lg[:], lhsT=xT[:, dci, :], rhs=rw_sb[:, dci, :],
                                         start=(dci == 0), stop=(dci == DC - 1))
                    lsb = rt.tile([P, E], f32, tag="lsb")
                    nc.vector.tensor_tensor(out=lsb[:], in0=lg[:], in1=rb_sb[:],
                                            op=mybir.AluOpType.add)
                    mx = rt.tile([P, 8], f32, tag="mx")
                    mi = rt.tile([P, 8], mybir.dt.uint32, tag="mi")
                    nc.vector.max_with_indices(mx[:], mi[:], lsb[:])
                    diff = rt.tile([P, 1], f32, tag="diff")
                    nc.vector.tensor_tensor(out=diff[:], in0=mx[:, 0:1], in1=mx[:, 1:2],
                                            op=mybir.AluOpType.subtract)
                    g1 = rt.tile([P, 1], f32, tag="g1")
                    nc.scalar.activation(out=g1[:], in_=diff[:],
                                         func=mybir.ActivationFunctionType.Sigmoid)
                    # fields: E1 | E2 | G1 | G2 at cols g, 8+g, 16+g, 24+g
                    nc.vector.tensor_copy(meta_sb[:, g:g + 1], mi[:, 0:1])
                    nc.vector.tensor_copy(meta_sb[:, E + g:E + g + 1], mi[:, 1:2])
                    nc.vector.tensor_copy(meta_sb[:, 2 * E + g:2 * E + g + 1], g1[:])
                    nc.vector.tensor_scalar(out=meta_sb[:, 3 * E + g:3 * E + g + 1],
                                            in0=g1[:], scalar1=-1.0, scalar2=1.0,
                                            op0=mybir.AluOpType.mult,
                                            op1=mybir.AluOpType.add)
                nc.sync.dma_start(out=meta_in.ap(), in_=meta_sb[:])

            # ---------- metadata AllGather ----------
            nc.gpsimd.collective_compute(
                "AllGather", mybir.AluOpType.bypass,
                replica_groups=[list(range(N_CORES))],
                ins=[meta_in.ap().opt()], outs=[meta_all.ap().opt()],
            )

            # ---------- global routing arrays ----------
            with tc.tile_pool(name="mt", bufs=1) as mt, \
                 tc.tile_pool(name="mtp", bufs=2, space="PSUM") as mtp:
                mload = mt.tile([P, N_CORES, 4 * E], f32)
                nc.sync.dma_start(out=mload[:], in_=meta_all.ap().rearrange(
                    "r p w -> p r w"))
                E1 = mt.tile([P, NCOL], f32)
                E2 = mt.tile([P, NCOL], f32)
                G1 = mt.tile([P, NCOL], f32)
                G2 = mt.tile([P, NCOL], f32)
                for fld, dst in ((0, E1), (1, E2), (2, G1), (3, G2)):
                    nc.vector.tensor_copy(dst[:], mload[:, :, fld * E:(fld + 1) * E])

                ranks = []          # per-expert exclusive global rank [P, NCOL]
                for e in range(E + 1):
                    mask = mt.tile([P, NCOL], f32, tag="mask", bufs=2)
                    if e < E:
                        m1 = mt.tile([P, NCOL], f32, tag="m1", bufs=2)
                        m2 = mt.tile([P, NCOL], f32, tag="m2", bufs=2)
                        nc.vector.tensor_scalar(out=m1[:], in0=E1[:], scalar1=float(e),
                                                scalar2=None, op0=mybir.AluOpType.is_equal)
                        nc.vector.tensor_scalar(out=m2[:], in0=E2[:], scalar1=float(e),
                                                scalar2=None, op0=mybir.AluOpType.is_equal)
                    else:
                        m1 = mt.tile([P, NCOL], f32, tag="m1", bufs=2)
                        m2 = mt.tile([P, NCOL], f32, tag="m2", bufs=2)
                        nc.vector.tensor_scalar(out=m1[:], in0=E1[:], scalar1=mye_sb[:, 0:1],
                                                scalar2=None, op0=mybir.AluOpType.is_equal)
                        nc.vector.tensor_scalar(out=m2[:], in0=E2[:], scalar1=mye_sb[:, 0:1],
                                                scalar2=None, op0=mybir.AluOpType.is_equal)
                    nc.vector.tensor_tensor(out=mask[:], in0=m1[:], in1=m2[:],
                                            op=mybir.AluOpType.add)
                    # within-column exclusive prefix (over partitions)
                    rps = mtp.tile([P, NCOL], f32, space="PSUM", tag="rps")
                    nc.tensor.matmul(out=rps[:], lhsT=tri[:], rhs=mask[:],
                                     start=True, stop=False)
                    # column totals -> exclusive cumsum across columns
                    cps = mtp.tile([1, NCOL], f32, space="PSUM", tag="cps")
                    nc.tensor.matmul(out=cps[:], lhsT=ones_col[:], rhs=mask[:],
                                     start=True, stop=True)
                    ctot = mt.tile([1, NCOL], f32, tag="ctot", bufs=2)
                    nc.vector.tensor_copy(ctot[:], cps[:])
                    cinc = mt.tile([1, NCOL], f32, tag="cinc", bufs=2)
                    nc.vector.tensor_tensor_scan(out=cinc[:], data0=ctot[:], data1=ctot[:],
                                                 initial=0.0, op0=mybir.AluOpType.add,
                                                 op1=mybir.AluOpType.bypass)
                    cexc = mt.tile([1, NCOL], f32, tag="cexc", bufs=2)
                    nc.vector.tensor_tensor(out=cexc[:], in0=cinc[:], in1=ctot[:],
                                            op=mybir.AluOpType.subtract)
                    # broadcast add into the same psum (accumulate)
                    nc.tensor.matmul(out=rps[:], lhsT=ones_row1[:], rhs=cexc[:],
                                     start=False, stop=True)
                    rk = mt.tile([P, NCOL], f32, name=f"rank_{e}", bufs=1)
                    nc.vector.tensor_copy(rk[:], rps[:])
                    ranks.append((rk, m1, m2, mask))

                # ----- combine index table (SPMD-uniform) -----
                gidxf = mt.tile([P, 2 * NCOL], f32)
                acc0 = mt.tile([P, NCOL], f32)
                acc1 = mt.tile([P, NCOL], f32)
                nc.vector.memset(acc0[:], 0.0)
                nc.vector.memset(acc1[:], 0.0)
                sel = mt.tile([P, NCOL], f32, tag="sel", bufs=2)
                for e in range(E):
                    rk, m1, m2, _ = ranks[e]
                    base = float(e * CAP)
                    # acc0 += m1 * (rank + e*CAP) ; acc1 += m2 * (rank + e*CAP)
                    nc.vector.tensor_scalar(out=sel[:], in0=rk[:], scalar1=base,
                                            scalar2=None, op0=mybir.AluOpType.add)
                    tmp = mt.tile([P, NCOL], f32, tag="tmp", bufs=2)
                    nc.vector.tensor_tensor(out=tmp[:], in0=sel[:], in1=m1[:],
                                            op=mybir.AluOpType.mult)
                    nc.vector.tensor_tensor(out=acc0[:], in0=acc0[:], in1=tmp[:],
                                            op=mybir.AluOpType.add)
                    nc.vector.tensor_tensor(out=tmp[:], in0=sel[:], in1=m2[:],
                                            op=mybir.AluOpType.mult)
                    nc.vector.tensor_tensor(out=acc1[:], in0=acc1[:], in1=tmp[:],
                                            op=mybir.AluOpType.add)
                # capacity drop -> dummy row.  kept_k = (rank_{Ek} < CAP)
                # rank under its own expert: rsel_k = acc_k - Ek*CAP... equal to rank.
                for acc, Ew in ((acc0, E1), (acc1, E2)):
                    rsel = mt.tile([P, NCOL], f32, tag="rsel", bufs=2)
                    nc.vector.tensor_scalar(out=rsel[:], in0=Ew[:], scalar1=float(CAP),
                                            scalar2=None, op0=mybir.AluOpType.mult)
                    nc.vector.tensor_tensor(out=rsel[:], in0=acc[:], in1=rsel[:],
                                            op=mybir.AluOpType.subtract)  # = rank
                    kept = mt.tile([P, NCOL], f32, tag="kept", bufs=2)
                    nc.vector.tensor_scalar(out=kept[:], in0=rsel[:], scalar1=float(CAP),
                                            scalar2=None, op0=mybir.AluOpType.is_lt)
                    # acc = kept ? acc : DUMMY_ROW
                    nc.vector.tensor_tensor(out=acc[:], in0=acc[:], in1=kept[:],
                                            op=mybir.AluOpType.mult)
                    nc.vector.tensor_scalar(out=kept[:], in0=kept[:],
                                            scalar1=-float(DUMMY_ROW),
                                            scalar2=float(DUMMY_ROW),
                                            op0=mybir.AluOpType.mult,
                                            op1=mybir.AluOpType.add)
                    nc.vector.tensor_tensor(out=acc[:], in0=acc[:], in1=kept[:],
                                            op=mybir.AluOpType.add)
                # interleave (idx0, idx1) and dump p-major
                nc.vector.tensor_copy(gidxf[:, 0:2 * NCOL:2], acc0[:])
                nc.vector.tensor_copy(gidxf[:, 1:2 * NCOL:2], acc1[:])
                gidxi = mt.tile([P, 2 * NCOL], i32)
                nc.gpsimd.tensor_copy(gidxi[:], gidxf[:])
                nc.gpsimd.dma_start(out=gidx_d.ap(), in_=gidxi[:])

                # ----- my expert: gate weights + remapped slot scatter -----
                rk_m, m1_m, m2_m, mask_m = ranks[E]
                wmine = mt.tile([P, NCOL], f32)
                tmpw = mt.tile([P, NCOL], f32, tag="tmp", bufs=2)
                nc.vector.tensor_tensor(out=wmine[:], in0=m1_m[:], in1=G1[:],
                                        op=mybir.AluOpType.mult)
                nc.vector.tensor_tensor(out=tmpw[:], in0=m2_m[:], in1=G2[:],
                                        op=mybir.AluOpType.mult)
                nc.vector.tensor_tensor(out=wmine[:], in0=wmine[:], in1=tmpw[:],
                                        op=mybir.AluOpType.add)
                # kept & dst' = (r & 127)*NSLOT_T + (r >> 7); not-kept -> OOB
                keptm = mt.tile([P, NCOL], f32, tag="kept", bufs=2)
                nc.vector.tensor_scalar(out=keptm[:], in0=rk_m[:], scalar1=float(CAP),
                                        scalar2=None, op0=mybir.AluOpType.is_lt)
                nc.vector.tensor_tensor(out=keptm[:], in0=keptm[:], in1=mask_m[:],
                                        op=mybir.AluOpType.mult)
                rki = mt.tile([P, NCOL], i32)
                nc.vector.tensor_copy(rki[:], rk_m[:])
                rand_ = mt.tile([P, NCOL], i32, tag="ri1", bufs=1)
                rshr = mt.tile([P, NCOL], i32, tag="ri2", bufs=1)
                nc.vector.tensor_scalar(out=rand_[:], in0=rki[:], scalar1=127,
                                        scalar2=None, op0=mybir.AluOpType.bitwise_and)
                nc.vector.tensor_scalar(out=rand_[:], in0=rand_[:], scalar1=NSLOT_T,
                                        scalar2=None, op0=mybir.AluOpType.mult)
                nc.vector.tensor_scalar(out=rshr[:], in0=rki[:], scalar1=7,
                                        scalar2=None,
                                        op0=mybir.AluOpType.logical_shift_right)
                dstp = mt.tile([P, NCOL], i32)
                nc.vector.tensor_tensor(out=dstp[:], in0=rand_[:], in1=rshr[:],
                                        op=mybir.AluOpType.add)
                # push non-kept OOB: dst += (1-kept)*8192
                oob = mt.tile([P, NCOL], f32, tag="tmp", bufs=2)
                nc.vector.tensor_scalar(out=oob[:], in0=keptm[:], scalar1=-8192.0,
                                        scalar2=8192.0, op0=mybir.AluOpType.mult,
                                        op1=mybir.AluOpType.add)
                oobi = mt.tile([P, NCOL], i32, tag="ri3", bufs=1)
                nc.vector.tensor_copy(oobi[:], oob[:])
                nc.vector.tensor_tensor(out=dstp[:], in0=dstp[:], in1=oobi[:],
                                        op=mybir.AluOpType.add)
                # move offsets + payload near gpsimd
                dstp_g = mt.tile([P, NCOL], i32)
                nc.gpsimd.tensor_copy(dstp_g[:], dstp[:])
                pay = mt.tile([P, 2 * NCOL], f32)
                nc.gpsimd.tensor_copy(pay[:, 0:2 * NCOL:2], tokf[:])
                nc.gpsimd.tensor_copy(pay[:, 1:2 * NCOL:2], wmine[:])
                for c in range(NCOL):
                    nc.gpsimd.indirect_dma_start(
                        out=slotmeta.ap(),
                        out_offset=bass.IndirectOffsetOnAxis(ap=dstp_g[:, c:c + 1], axis=0),
                        in_=pay[:, 2 * c:2 * c + 2],
                        in_offset=None,
                        bounds_check=CAP - 1,
                        oob_is_err=False,
                    )

            # ---------- load compacted slot meta ----------
            slot_tok = cpool.tile([P, NSLOT_T], i32)
            slot_w = cpool.tile([P, NSLOT_T], f32)
            smf = cpool.tile([P, NSLOT_T, 2], f32)
            nc.sync.dma_start(out=smf[:], in_=slotmeta.ap().rearrange(
                "(p s) w -> p s w", p=P))
            nc.vector.tensor_copy(slot_w[:], smf[:, :, 1])
            slot_tokg = cpool.tile([P, NSLOT_T], f32)
            nc.gpsimd.tensor_copy(slot_tokg[:], smf[:, :, 0])
            nc.gpsimd.tensor_copy(slot_tok[:], slot_tokg[:])

            # ---------- combine index prefetch (overlaps FFN) ----------
            gi_tiles = []
            for g in range(TPC // P):
                myr = cpool.tile([P, 1], i32, name=f"myr_{g}")
                nc.gpsimd.dma_start(out=myr[:], in_=my_rows.ap()[:, g:g + 1])
                gi = cpool.tile([P, 2], i32, name=f"gi_{g}")
                nc.gpsimd.indirect_dma_start(
                    out=gi[:], out_offset=None,
                    in_=gidx_d.ap().rearrange("p (r w) -> (p r) w", w=2),
                    in_offset=bass.IndirectOffsetOnAxis(ap=myr[:], axis=0),
                )
                gi_tiles.append(gi)

            # =========== expert FFN ===========
            with tc.tile_pool(name="wts", bufs=1) as wts:
                w1_sb = wts.tile([P, DC, FC, P], f32r)
                # w1[dci*128+p, fci*128+fc] -> [p, dci, fci, fc]
                nc.gpsimd.dma_start(out=w1_sb[:], in_=w1_p.ap().rearrange(
                    "(dc p) (fc q) -> p dc fc q", p=P, q=P))
                b1_sb = wts.tile([P, FC], f32)
                nc.sync.dma_start(out=b1_sb[:], in_=b1_p.ap())
                b2_sb = wts.tile([P, DC], f32)
                nc.sync.dma_start(out=b2_sb[:], in_=b2_p.ap())

                with tc.tile_pool(name="ffn", bufs=1) as ffn, \
                     tc.tile_pool(name="ffg", bufs=2) as ffg, \
                     tc.tile_pool(name="ffp", bufs=2, space="PSUM") as ffp:
                    for g in range(NGRP):
                        xgT = ffn.tile([P, DC, GRP], f32r, tag="xgT")
                        for st in range(GRP // P):
                            s = g * (GRP // P) + st
                            xg = ffg.tile([P, D], f32, tag="xg", bufs=3)
                            nc.gpsimd.indirect_dma_start(
                                out=xg[:], out_offset=None,
                                in_=x_full.ap(),
                                in_offset=bass.IndirectOffsetOnAxis(
                                    ap=slot_tok[:, s:s + 1], axis=0),
                                bounds_check=T - 1,
                                oob_is_err=False,
                            )
                            for dci in range(DC):
                                tp = ffp.tile([P, P], f32, space="PSUM", tag="tp")
                                nc.tensor.transpose(out=tp[:],
                                                    in_=xg[:, dci * P:(dci + 1) * P],
                                                    identity=ident[:])
                                nc.vector.tensor_copy(
                                    xgT[:, dci, st * P:(st + 1) * P], tp[:])
                        # mm1 + gelu -> hT
                        hT = ffn.tile([P, FC, GRP], f32r, tag="hT")
                        for fci in range(FC):
                            hp = ffp.tile([P, GRP], f32, space="PSUM", tag="hp")
                            for dci in range(DC):
                                nc.tensor.matmul(out=hp[:],
                                                 lhsT=w1_sb[:, dci, fci, :],
                                                 rhs=xgT[:, dci, :],
                                                 start=(dci == 0), stop=(dci == DC - 1))
                            nc.scalar.activation(out=hT[:, fci, :], in_=hp[:],
                                                 func=mybir.ActivationFunctionType.Gelu,
                                                 bias=b1_sb[:, fci:fci + 1], scale=1.0)
                        # mm2 (+bias) -> oT
                        oT = ffn.tile([P, DC, GRP], f32, tag="oT")
                        for dci in range(DC):
                            w2c = ffg.tile([P, FC, P], f32r, tag="w2c", bufs=2)
                            nc.gpsimd.dma_start(out=w2c[:], in_=w2_p.ap().rearrange(
                                "(fc p) (dc q) -> p fc dc q", p=P, q=P)[:, :, dci, :])
                            op = ffp.tile([P, GRP], f32, space="PSUM", tag="op")
                            for fci in range(FC):
                                nc.tensor.matmul(out=op[:],
                                                 lhsT=w2c[:, fci, :],
                                                 rhs=hT[:, fci, :],
                                                 start=(fci == 0), stop=(fci == FC - 1))
                            nc.vector.tensor_scalar(out=oT[:, dci, :], in0=op[:],
                                                    scalar1=b2_sb[:, dci:dci + 1],
                                                    scalar2=None,
                                                    op0=mybir.AluOpType.add)
                        # transpose back, gate, store
                        for st in range(GRP // P):
                            s = g * (GRP // P) + st
                            ow = ffg.tile([P, D], f32, tag="ow", bufs=3)
                            for dci in range(DC):
                                tp2 = ffp.tile([P, P], f32, space="PSUM", tag="tp2")
                                nc.tensor.transpose(out=tp2[:],
                                                    in_=oT[:, dci, st * P:(st + 1) * P],
                                                    identity=ident[:])
                                nc.scalar.activation(out=ow[:, dci * P:(dci + 1) * P],
                                                     in_=tp2[:],
                                                     func=mybir.ActivationFunctionType.Copy,
                                                     scale=slot_w[:, s:s + 1])
                            nc.sync.dma_start(out=eo_d.ap()[s * P:(s + 1) * P, :],
                                              in_=ow[:])

            # =========== combine ===========
            nc.gpsimd.collective_compute(
                "AllGather", mybir.AluOpType.bypass,
                replica_groups=[list(range(N_CORES))],
                ins=[eo_d.ap().opt()], outs=[eo_all.ap()[:E * CAP, :].opt()],
            )
            with tc.tile_pool(name="cmb", bufs=3) as cmb:
                zrow = cmb.tile([P, D], f32, bufs=1)
                nc.vector.memset(zrow[:], 0.0)
                nc.sync.dma_start(out=eo_all.ap()[DUMMY_ROW:DUMMY_ROW + P, :], in_=zrow[:])
                for g in range(TPC // P):
                    gi = gi_tiles[g]
                    cb0 = cmb.tile([P, D], f32, tag="cb0")
                    cb1 = cmb.tile([P, D], f32, tag="cb1")
                    nc.gpsimd.indirect_dma_start(
                        out=cb0[:], out_offset=None, in_=eo_all.ap(),
                        in_offset=bass.IndirectOffsetOnAxis(ap=gi[:, 0:1], axis=0),
                    )
                    nc.gpsimd.indirect_dma_start(
                        out=cb1[:], out_offset=None, in_=eo_all.ap(),
                        in_offset=bass.IndirectOffsetOnAxis(ap=gi[:, 1:2], axis=0),
                    )
                    osb = cmb.tile([P, D], f32, tag="osb")
                    nc.vector.tensor_tensor(out=osb[:], in0=cb0[:], in1=cb1[:],
                                            op=mybir.AluOpType.add)
                    nc.sync.dma_start(out=out_shard.ap()[g * P:(g + 1) * P, :], in_=osb[:])

    nc.finalize()
    return nc


_NC_CACHE = None
TRACE = False
LAST_EXEC_NS = None
LAST_TRACE_DIR = None


def kernel(x, router_w, router_b, w1, b1, w2, b2):
    global _NC_CACHE
    x = np.ascontiguousarray(np.asarray(x, np.float32))
    router_w = np.ascontiguousarray(np.asarray(router_w, np.float32))
    router_b = np.asarray(router_b, np.float32)
    w1 = np.asarray(w1, np.float32)
    b1 = np.asarray(b1, np.float32)
    w2 = np.asarray(w2, np.float32)
    b2 = np.asarray(b2, np.float32)

    xf = x.reshape(T, D)
    rb_b = np.tile(router_b[None, :], (P, 1))

    in_maps = []
    for c in range(N_CORES):
        toks = np.arange(c * TPC, (c + 1) * TPC)
        my_rows = ((toks % P) * NCOL + toks // P).astype(np.int32)
        in_maps.append({
            "x_full": xf,
            "x_shard": np.ascontiguousarray(xf[c * TPC:(c + 1) * TPC]),
            "rw": router_w,
            "rb_b": rb_b,
            "w1_p": np.ascontiguousarray(w1[c]),
            "b1_p": np.ascontiguousarray(b1[c].reshape(FC, P).T),
            "w2_p": np.ascontiguousarray(w2[c]),
            "b2_p": np.ascontiguousarray(b2[c].reshape(DC, P).T),
            "my_e": np.full((P, 1), float(c), np.float32),
            "my_rows": np.ascontiguousarray(my_rows.reshape(TPC // P, P).T),
        })

    global LAST_EXEC_NS, LAST_TRACE_DIR
    if _NC_CACHE is None:
        _NC_CACHE = build_kernel()
    import tempfile
    td = tempfile.mkdtemp(prefix="moe_trace_") if TRACE else None
    res = run_bass_kernel_spmd(_NC_CACHE, in_maps, list(range(N_CORES)),
                               trace=TRACE, tmpdir=td)
    LAST_EXEC_NS = getattr(res, "exec_time_ns", None)
    LAST_TRACE_DIR = td
    out = np.concatenate([res.results[c]["out_shard"] for c in range(N_CORES)], axis=0)
    return out.reshape(B, S, D)
